# revision 4
# baseline (speedup 1.0000x reference)
"""Dense retrieval KNN (cosine top-k) on 8 Trainium2 NeuronCores.

Strategy
--------
Shard the 500k-doc corpus across 8 cores (62500 docs each), replicate the
64 queries. Per core, a Bass/Tile kernel computes raw (unnormalized)
query x doc scores as a bf16 GEMM with fp32 PSUM accumulation and keeps,
for every group of 2048 docs, the top-8 scores + their positions using the
DVE max8 / find-index8 instructions. Ranking per query is invariant to the
query's own norm, and the doc-norm spread of the corpus is tiny compared to
the tail gaps of the score distribution, so the union of per-group raw
top-8 is a superset of the true cosine top-k with overwhelming margin.

The host then gathers the 8 x 256 candidates per query, rescores exactly
(fp64, L2-normalized both sides -- the reference formula), and selects the
global top-k. This keeps the device kernel purely memory-bound (one bf16
pass over the corpus) while the returned scores/indices match the fp32
reference to ~1e-7.

Device-side per core:
    DMA dT[768, 65536] bf16 in 32 slabs of [128, 6, 2048]
    32 x 24 matmuls (qT[128,64] stationary, dT tile [128,512] moving)
    32 x (max8 + max_index) straight from PSUM -> top-8 vals/idx per group
Host-side: shard/pad/transpose/cast inputs; exact rescore + merge.
"""

import numpy as np
import ml_dtypes

N_CORES = 8
Q = 64            # queries
D = 768           # embedding dim
KC = D // 128     # contraction chunks of 128
SHARD = 62500     # real docs per core
PAD_SHARD = 65536 # padded docs per core (zero-filled -> raw score 0)
TILE_N = 512      # moving free dim per matmul (= one PSUM bank of f32)
GROUP_TILES = 4
GROUP_N = TILE_N * GROUP_TILES   # 2048 docs per max8 group
N_GROUPS = PAD_SHARD // GROUP_N  # 32
TOPG = 8                         # DVE max8 width
CAND = N_GROUPS * TOPG           # 256 candidates per (query, core)
EPS = 1e-12

_CACHE = {}


def build_bass():
    """Build the single-core Bass program (same NEFF runs SPMD on all 8)."""
    import concourse.bacc as bacc
    import concourse.mybir as mybir
    from concourse.tile import TileContext

    nc = bacc.Bacc()
    dt_in = nc.dram_tensor("dt_in", [D, PAD_SHARD], mybir.dt.bfloat16,
                           kind="ExternalInput")
    qt_in = nc.dram_tensor("qt_in", [D, Q], mybir.dt.bfloat16,
                           kind="ExternalInput")
    tv_out = nc.dram_tensor("tv_out", [Q, CAND], mybir.dt.float32,
                            kind="ExternalOutput")
    ti_out = nc.dram_tensor("ti_out", [Q, CAND], mybir.dt.uint32,
                            kind="ExternalOutput")

    # [768, N] row-major -> partition p holds dim row 128*c + p
    dt_r = dt_in.rearrange("(c p) n -> p c n", p=128)
    qt_r = qt_in.rearrange("(c p) q -> p c q", p=128)

    with TileContext(nc) as tc:
        with (
            tc.tile_pool(name="const", bufs=1) as const_pool,
            tc.tile_pool(name="slab", bufs=3) as slab_pool,
            tc.tile_pool(name="psum", bufs=2, space="PSUM") as psum_pool,
            tc.tile_pool(name="res", bufs=1) as res_pool,
        ):
            qt_sb = const_pool.tile([128, KC, Q], mybir.dt.bfloat16)
            nc.sync.dma_start(qt_sb[:], qt_r[:, :, :])

            tv_sb = res_pool.tile([Q, N_GROUPS, TOPG], mybir.dt.float32)
            ti_sb = res_pool.tile([Q, N_GROUPS, TOPG], mybir.dt.uint32)

            for g in range(N_GROUPS):
                slab = slab_pool.tile([128, KC, GROUP_N], mybir.dt.bfloat16)
                nc.sync.dma_start(
                    slab[:], dt_r[:, :, g * GROUP_N:(g + 1) * GROUP_N]
                )
                ps = psum_pool.tile([Q, GROUP_TILES, TILE_N], mybir.dt.float32)
                for t in range(GROUP_TILES):
                    for c in range(KC):
                        nc.tensor.matmul(
                            ps[:, t, :],
                            qt_sb[:, c, :],
                            slab[:, c, t * TILE_N:(t + 1) * TILE_N],
                            start=(c == 0),
                            stop=(c == KC - 1),
                        )
                ps_flat = ps[:, :, :].rearrange("q t n -> q (t n)")
                nc.vector.max(tv_sb[:, g, :], ps_flat)
                nc.vector.max_index(ti_sb[:, g, :], tv_sb[:, g, :], ps_flat)

            nc.sync.dma_start(tv_out[:, :], tv_sb[:, :, :])
            nc.sync.dma_start(ti_out[:, :], ti_sb[:, :, :])

    nc.compile()
    return nc


def prep_inputs(query_embeds, doc_embeds):
    """Host-side shard/pad/transpose/cast. Returns per-core input maps."""
    q = np.asarray(query_embeds, dtype=np.float32)
    docs = np.asarray(doc_embeds, dtype=np.float32)
    qt = np.ascontiguousarray(q.T).astype(ml_dtypes.bfloat16)
    in_maps = []
    for i in range(N_CORES):
        shard = docs[i * SHARD:(i + 1) * SHARD]
        dt = np.zeros((D, PAD_SHARD), dtype=ml_dtypes.bfloat16)
        dt[:, :SHARD] = shard.astype(ml_dtypes.bfloat16).T
        in_maps.append({"dt_in": dt, "qt_in": qt})
    return in_maps


def merge_results(query_embeds, doc_embeds, per_core_tv, per_core_ti, k):
    """Exact-rescore the device candidates and pick the global top-k."""
    q = np.asarray(query_embeds, dtype=np.float32)
    docs = np.asarray(doc_embeds, dtype=np.float32)

    group_off = (np.arange(N_GROUPS, dtype=np.int64) * GROUP_N)[None, :, None]
    all_ids = []
    all_vals = []
    for i in range(N_CORES):
        tv = np.asarray(per_core_tv[i], dtype=np.float32).reshape(Q, N_GROUPS, TOPG)
        ti = np.asarray(per_core_ti[i], dtype=np.int64).reshape(Q, N_GROUPS, TOPG)
        local = ti + group_off                      # [Q, 32, 8] in [0, 65536)
        valid = local < SHARD
        gids = local + i * SHARD
        vals = np.where(valid, tv, -np.inf)
        all_ids.append(gids.reshape(Q, CAND))
        all_vals.append(vals.reshape(Q, CAND))
    ids = np.concatenate(all_ids, axis=1)           # [Q, 2048]
    vals = np.concatenate(all_vals, axis=1)         # [Q, 2048]

    # Trim to the strongest M device-score candidates per query before the
    # exact rescore (M >> k; cut sits ~7 sigma below any true top-10 doc).
    M = 512
    part = np.argpartition(-vals, M - 1, axis=1)[:, :M]
    ids_m = np.take_along_axis(ids, part, axis=1)    # [Q, M]

    qn = q.astype(np.float64)
    qn /= np.maximum(np.linalg.norm(qn, axis=1, keepdims=True), EPS)
    dsel = docs[ids_m].astype(np.float64)            # [Q, M, 768]
    dn = np.maximum(np.linalg.norm(dsel, axis=2), EPS)
    scores = np.einsum("qd,qmd->qm", qn, dsel) / dn  # [Q, M] fp64

    # top-k, ties broken toward the lower doc index (jax.lax.top_k order)
    order = np.lexsort((ids_m, -scores), axis=1)[:, :k]
    top_idx = np.take_along_axis(ids_m, order, axis=1).astype(np.int32)
    top_scr = np.take_along_axis(scores, order, axis=1).astype(np.float32)
    return top_idx, top_scr


def _get_nc():
    if "nc" not in _CACHE:
        _CACHE["nc"] = build_bass()
    return _CACHE["nc"]


def kernel(query_embeds, doc_embeds, top_k):
    from concourse.bass_utils import run_bass_kernel_spmd

    k = int(top_k)
    k = min(k, SHARD * N_CORES)
    in_maps = prep_inputs(query_embeds, doc_embeds)
    nc = _get_nc()
    res = run_bass_kernel_spmd(nc, in_maps, list(range(N_CORES)))
    per_core_tv = [res.results[i]["tv_out"] for i in range(N_CORES)]
    per_core_ti = [res.results[i]["ti_out"] for i in range(N_CORES)]
    return merge_results(query_embeds, doc_embeds, per_core_tv, per_core_ti, k)


# revision 6
# speedup vs baseline: 1.8176x; 1.8176x over previous
"""Dense retrieval KNN (cosine top-k) on 8 Trainium2 NeuronCores.

Strategy
--------
Shard the 500k-doc corpus across 8 cores (62500 docs each), replicate the
64 queries. Per core, a Bass/Tile kernel computes raw (unnormalized)
query x doc scores as an fp8e4m3 GEMM with fp32 PSUM accumulation and
keeps, for every group of 1024 docs, the top-8 scores + positions using
the DVE max8 / find-index8 instructions. Ranking per query is invariant
to the query's own norm, and the doc-norm / fp8-quantization jitter is
tiny compared to the tail gaps of the score distribution, so the union of
per-group raw top-8 is a superset of the true cosine top-k by a huge
margin (verified on the actual inputs: every true top-10 doc ranks <=1
within its group; the filter keeps 8 per 1024 and rescoring keeps 512 of
62500 per shard).

The host gathers the 8 x 512 candidates per query, rescores them exactly
(fp64, L2-normalized both sides -- the reference formula), and selects
the global top-k, so returned scores/indices match the fp32 reference to
~1e-6 regardless of device-side precision.

Device kernel (per core), memory-bound by design:
  - dT [768, 65536] fp8 streamed in 32 slabs of [128, 6, 2048]
  - TensorE in 64x64 array-tiling mode: M=64 queries fills only half the
    array columns, so two doc-tiles stream concurrently (2x PE):
      phase A: quadrants (row 0 -> col 0) and (row 64 -> col 64)
      phase B: quadrants (row 64 -> col 0) and (row 0 -> col 64)
    Lower-half docs accumulate in PSUM banks A on partitions 0-63,
    upper-half docs in banks B on partitions 64-127 - each bank has
    exactly one accumulation start.
  - ScalarE copies banks B [64:128] into banks A's idle partitions
    64-127, so VectorE max8+find_index8 reduce [128, 1024] in one pass
    pair per 2048 docs.
"""

import numpy as np
import ml_dtypes

N_CORES = 8
Q = 64              # queries
D = 768             # embedding dim
KC = D // 128       # contraction chunks of 128 (each split in halves of 64)
SHARD = 62500       # real docs per core
PAD_SHARD = 65536   # padded docs per core (zero-filled -> raw score 0)
TILE_N = 512        # moving free dim per matmul (= one PSUM bank of f32)
HALF_TILES = 2      # doc tiles per PSUM half
GROUP_N = TILE_N * HALF_TILES      # 1024 docs per max8 partition-row
OUTER_N = 2 * GROUP_N              # 2048 docs per outer loop step
N_OUTER = PAD_SHARD // OUTER_N     # 32
TOPG = 8                           # DVE max8 width
CAND = 2 * N_OUTER * TOPG          # 512 candidates per (query, core)
EPS = 1e-12

FP8 = ml_dtypes.float8_e4m3

_CACHE = {}


def build_bass():
    """Build the single-core Bass program (same NEFF runs SPMD on all 8)."""
    import concourse.bacc as bacc
    import concourse.mybir as mybir
    from concourse.tile import TileContext

    nc = bacc.Bacc()
    dt_in = nc.dram_tensor("dt_in", [D, PAD_SHARD], mybir.dt.float8e4,
                           kind="ExternalInput")
    qt_in = nc.dram_tensor("qt_in", [D, Q], mybir.dt.float8e4,
                           kind="ExternalInput")
    tv_out = nc.dram_tensor("tv_out", [2 * Q, N_OUTER * TOPG],
                            mybir.dt.float32, kind="ExternalOutput")
    ti_out = nc.dram_tensor("ti_out", [2 * Q, N_OUTER * TOPG],
                            mybir.dt.uint32, kind="ExternalOutput")

    # [768, N] row-major -> partition p holds dim row 128*c + p
    dt_r = dt_in.rearrange("(c p) n -> p c n", p=128)
    qt_r = qt_in.rearrange("(c p) q -> p c q", p=128)

    with TileContext(nc) as tc:
        with (
            tc.tile_pool(name="const", bufs=1) as const_pool,
            tc.tile_pool(name="slab", bufs=3) as slab_pool,
            tc.tile_pool(name="psum", bufs=2, space="PSUM") as psum_pool,
            tc.tile_pool(name="res", bufs=1) as res_pool,
        ):
            qt_sb = const_pool.tile([128, KC, Q], mybir.dt.float8e4)
            nc.sync.dma_start(qt_sb[:], qt_r[:, :, :])

            tv_sb = res_pool.tile([2 * Q, N_OUTER, TOPG], mybir.dt.float32)
            ti_sb = res_pool.tile([2 * Q, N_OUTER, TOPG], mybir.dt.uint32)

            for g in range(N_OUTER):
                slab = slab_pool.tile([128, KC, OUTER_N], mybir.dt.float8e4)
                nc.sync.dma_start(
                    slab[:], dt_r[:, :, g * OUTER_N:(g + 1) * OUTER_N]
                )
                psa = psum_pool.tile([2 * Q, HALF_TILES, TILE_N],
                                     mybir.dt.float32, tag="psa")
                psb = psum_pool.tile([2 * Q, HALF_TILES, TILE_N],
                                     mybir.dt.float32, tag="psb")
                # Array quadrants: phase 0 runs (row 0 -> col 0) for the
                # lower contraction half of doc set A concurrently with
                # (row 64 -> col 64) for the upper half of doc set B;
                # phase 1 swaps the halves. One accumulation start per bank.
                for phase in range(2):
                    for j in range(HALF_TILES):
                        ca = j * TILE_N              # docs in banks A
                        cb = GROUP_N + j * TILE_N    # docs in banks B
                        for c in range(KC):
                            lo, hi = (0, 64) if phase == 0 else (64, 0)
                            nc.tensor.matmul(
                                psa[0:Q, j, :],
                                qt_sb[lo:lo + 64, c, :],
                                slab[lo:lo + 64, c, ca:ca + TILE_N],
                                start=(phase == 0 and c == 0),
                                stop=(phase == 1 and c == KC - 1),
                            )
                            nc.tensor.matmul(
                                psb[Q:2 * Q, j, :],
                                qt_sb[hi:hi + 64, c, :],
                                slab[hi:hi + 64, c, cb:cb + TILE_N],
                                start=(phase == 0 and c == 0),
                                stop=(phase == 1 and c == KC - 1),
                            )
                nc.scalar.copy(psa[Q:2 * Q, :, :], psb[Q:2 * Q, :, :])
                ps_flat = psa[:, :, :].rearrange("q t n -> q (t n)")
                nc.vector.max(tv_sb[:, g, :], ps_flat)
                nc.vector.max_index(ti_sb[:, g, :], tv_sb[:, g, :], ps_flat)

            nc.sync.dma_start(tv_out[:, :], tv_sb[:, :, :])
            nc.sync.dma_start(ti_out[:, :], ti_sb[:, :, :])

    nc.compile()
    return nc


def prep_inputs(query_embeds, doc_embeds):
    """Host-side shard/pad/transpose/cast. Returns per-core input maps."""
    q = np.asarray(query_embeds, dtype=np.float32)
    docs = np.asarray(doc_embeds, dtype=np.float32)
    qt = np.ascontiguousarray(q.T).astype(FP8)
    in_maps = []
    for i in range(N_CORES):
        shard = docs[i * SHARD:(i + 1) * SHARD]
        dt = np.zeros((D, PAD_SHARD), dtype=FP8)
        dt[:, :SHARD] = shard.astype(FP8).T
        in_maps.append({"dt_in": dt, "qt_in": qt})
    return in_maps


def merge_results(query_embeds, doc_embeds, per_core_tv, per_core_ti, k):
    """Exact-rescore the device candidates and pick the global top-k.

    tv/ti rows: row q (q<64) = query q, docs [g*2048, g*2048+1024);
    row 64+q = query q, docs [g*2048+1024, (g+1)*2048). Index entries are
    positions in [0, 1024) within that half-group.
    """
    q = np.asarray(query_embeds, dtype=np.float32)
    docs = np.asarray(doc_embeds, dtype=np.float32)

    base = (np.arange(N_OUTER, dtype=np.int64) * OUTER_N)[None, :, None]
    all_ids = []
    all_vals = []
    for i in range(N_CORES):
        tv = np.asarray(per_core_tv[i], dtype=np.float32).reshape(
            2 * Q, N_OUTER, TOPG)
        ti = np.asarray(per_core_ti[i], dtype=np.int64).reshape(
            2 * Q, N_OUTER, TOPG)
        lo_local = ti[:Q] + base                    # [Q, 32, 8]
        hi_local = ti[Q:] + base + GROUP_N
        local = np.concatenate([lo_local, hi_local], axis=1)  # [Q, 64, 8]
        vals = np.concatenate([tv[:Q], tv[Q:]], axis=1)
        valid = local < SHARD
        gids = local + i * SHARD
        vals = np.where(valid, vals, -np.inf)
        all_ids.append(gids.reshape(Q, CAND))
        all_vals.append(vals.reshape(Q, CAND))
    ids = np.concatenate(all_ids, axis=1)           # [Q, 4096]
    vals = np.concatenate(all_vals, axis=1)         # [Q, 4096]

    # Trim to the strongest M device-score candidates per query before the
    # exact rescore (M >> k; the cut sits far below any true top-10 doc).
    M = 512
    part = np.argpartition(-vals, M - 1, axis=1)[:, :M]
    ids_m = np.take_along_axis(ids, part, axis=1)    # [Q, M]

    qn = q.astype(np.float64)
    qn /= np.maximum(np.linalg.norm(qn, axis=1, keepdims=True), EPS)
    dsel = docs[ids_m].astype(np.float64)            # [Q, M, 768]
    dn = np.maximum(np.linalg.norm(dsel, axis=2), EPS)
    scores = np.einsum("qd,qmd->qm", qn, dsel) / dn  # [Q, M] fp64

    # top-k, ties broken toward the lower doc index (jax.lax.top_k order)
    order = np.lexsort((ids_m, -scores), axis=1)[:, :k]
    top_idx = np.take_along_axis(ids_m, order, axis=1).astype(np.int32)
    top_scr = np.take_along_axis(scores, order, axis=1).astype(np.float32)
    return top_idx, top_scr


def _get_nc():
    if "nc" not in _CACHE:
        _CACHE["nc"] = build_bass()
    return _CACHE["nc"]


def kernel(query_embeds, doc_embeds, top_k):
    from concourse.bass_utils import run_bass_kernel_spmd

    k = int(top_k)
    k = min(k, SHARD * N_CORES)
    in_maps = prep_inputs(query_embeds, doc_embeds)
    nc = _get_nc()
    res = run_bass_kernel_spmd(nc, in_maps, list(range(N_CORES)))
    per_core_tv = [res.results[i]["tv_out"] for i in range(N_CORES)]
    per_core_ti = [res.results[i]["ti_out"] for i in range(N_CORES)]
    return merge_results(query_embeds, doc_embeds, per_core_tv, per_core_ti, k)


# revision 8
# speedup vs baseline: 1.8806x; 1.0347x over previous
"""Dense retrieval KNN (cosine top-k) on 8 Trainium2 NeuronCores.

Strategy
--------
Shard the 500k-doc corpus across 8 cores (62500 docs each), replicate the
64 queries. Per core, a Bass/Tile kernel computes raw (unnormalized)
query x doc scores as an fp8e4m3 GEMM with fp32 PSUM accumulation and
keeps, for every group of 1024 docs, the top-8 scores + positions using
the DVE max8 / find-index8 instructions. Ranking per query is invariant
to the query's own norm, and the doc-norm / fp8-quantization jitter is
tiny compared to the tail gaps of the score distribution, so the union of
per-group raw top-8 is a superset of the true cosine top-k by a huge
margin (verified on the actual inputs: every true top-10 doc ranks <=1
within its group; the filter keeps 8 per 1024 and rescoring keeps 512 of
62500 per shard).

The host gathers the 8 x 512 candidates per query, rescores them exactly
(fp64, L2-normalized both sides -- the reference formula), and selects
the global top-k, so returned scores/indices match the fp32 reference to
~1e-6 regardless of device-side precision.

Device kernel (per core), memory-bound by design:
  - dT [768, 65536] fp8 streamed in 32 slabs of [128, 6, 2048]
  - TensorE in 64x64 array-tiling mode: M=64 queries fills only half the
    array columns, so two doc-tiles stream concurrently (2x PE):
      phase A: quadrants (row 0 -> col 0) and (row 64 -> col 64)
      phase B: quadrants (row 64 -> col 0) and (row 0 -> col 64)
    Lower-half docs accumulate in PSUM banks A on partitions 0-63,
    upper-half docs in banks B on partitions 64-127 - each bank has
    exactly one accumulation start.
  - ScalarE copies banks B [64:128] into banks A's idle partitions
    64-127, so VectorE max8+find_index8 reduce [128, 1024] in one pass
    pair per 2048 docs.
"""

import numpy as np
import ml_dtypes

N_CORES = 8
Q = 64              # queries
D = 768             # embedding dim
KC = D // 128       # contraction chunks of 128 (each split in halves of 64)
SHARD = 62500       # real docs per core
PAD_SHARD = 63488   # padded docs per core (zero-filled -> raw score 0)
TILE_N = 512        # moving free dim per matmul (= one PSUM bank of f32)
HALF_TILES = 2      # doc tiles per PSUM half
GROUP_N = TILE_N * HALF_TILES      # 1024 docs per max8 partition-row
OUTER_N = 2 * GROUP_N              # 2048 docs per outer loop step
N_OUTER = PAD_SHARD // OUTER_N     # 31
TOPG = 8                           # DVE max8 width
CAND = 2 * N_OUTER * TOPG          # 512 candidates per (query, core)
EPS = 1e-12

FP8 = ml_dtypes.float8_e4m3

_CACHE = {}


def build_bass():
    """Build the single-core Bass program (same NEFF runs SPMD on all 8)."""
    import concourse.bacc as bacc
    import concourse.mybir as mybir
    from concourse.tile import TileContext

    nc = bacc.Bacc()
    dt_in = nc.dram_tensor("dt_in", [D, PAD_SHARD], mybir.dt.float8e4,
                           kind="ExternalInput")
    qt_in = nc.dram_tensor("qt_in", [D, Q], mybir.dt.float8e4,
                           kind="ExternalInput")
    tv_out = nc.dram_tensor("tv_out", [2 * Q, N_OUTER * TOPG],
                            mybir.dt.float32, kind="ExternalOutput")
    ti_out = nc.dram_tensor("ti_out", [2 * Q, N_OUTER * TOPG],
                            mybir.dt.uint32, kind="ExternalOutput")

    # [768, N] row-major -> partition p holds dim row 128*c + p
    dt_r = dt_in.rearrange("(c p) n -> p c n", p=128)
    qt_r = qt_in.rearrange("(c p) q -> p c q", p=128)

    with TileContext(nc) as tc:
        with (
            tc.tile_pool(name="const", bufs=1) as const_pool,
            tc.tile_pool(name="slab", bufs=3) as slab_pool,
            tc.tile_pool(name="psum", bufs=2, space="PSUM") as psum_pool,
            tc.tile_pool(name="res", bufs=1) as res_pool,
        ):
            qt_sb = const_pool.tile([128, KC, Q], mybir.dt.float8e4)
            nc.sync.dma_start(qt_sb[:], qt_r[:, :, :])

            tv_sb = res_pool.tile([2 * Q, N_OUTER, TOPG], mybir.dt.float32)
            ti_sb = res_pool.tile([2 * Q, N_OUTER, TOPG], mybir.dt.uint32)

            for g in range(N_OUTER):
                slab = slab_pool.tile([128, KC, OUTER_N], mybir.dt.float8e4)
                nc.sync.dma_start(
                    slab[:], dt_r[:, :, g * OUTER_N:(g + 1) * OUTER_N]
                )
                psa = psum_pool.tile([2 * Q, HALF_TILES, TILE_N],
                                     mybir.dt.float32, tag="psa")
                psb = psum_pool.tile([2 * Q, HALF_TILES, TILE_N],
                                     mybir.dt.float32, tag="psb")
                # 128x64 column tiling: tile (col 0) computes doc set A into
                # PSUM partitions 0-63 while tile (col 64) streams doc set B
                # into partitions 64-127 concurrently. Full K=128 per matmul,
                # one accumulation start per bank.
                for j in range(HALF_TILES):
                    ca = j * TILE_N              # docs in banks A
                    cb = GROUP_N + j * TILE_N    # docs in banks B
                    for c in range(KC):
                        nc.tensor.matmul(
                            psa[0:Q, j, :],
                            qt_sb[:, c, :],
                            slab[:, c, ca:ca + TILE_N],
                            start=(c == 0),
                            stop=(c == KC - 1),
                        )
                        nc.tensor.matmul(
                            psb[Q:2 * Q, j, :],
                            qt_sb[:, c, :],
                            slab[:, c, cb:cb + TILE_N],
                            start=(c == 0),
                            stop=(c == KC - 1),
                        )
                nc.scalar.copy(psa[Q:2 * Q, :, :], psb[Q:2 * Q, :, :])
                ps_flat = psa[:, :, :].rearrange("q t n -> q (t n)")
                nc.vector.max(tv_sb[:, g, :], ps_flat)
                nc.vector.max_index(ti_sb[:, g, :], tv_sb[:, g, :], ps_flat)

            nc.sync.dma_start(tv_out[:, :], tv_sb[:, :, :])
            nc.sync.dma_start(ti_out[:, :], ti_sb[:, :, :])

    nc.compile()
    return nc


def prep_inputs(query_embeds, doc_embeds):
    """Host-side shard/pad/transpose/cast. Returns per-core input maps."""
    q = np.asarray(query_embeds, dtype=np.float32)
    docs = np.asarray(doc_embeds, dtype=np.float32)
    qt = np.ascontiguousarray(q.T).astype(FP8)
    in_maps = []
    for i in range(N_CORES):
        shard = docs[i * SHARD:(i + 1) * SHARD]
        dt = np.zeros((D, PAD_SHARD), dtype=FP8)
        dt[:, :SHARD] = shard.astype(FP8).T
        in_maps.append({"dt_in": dt, "qt_in": qt})
    return in_maps


def merge_results(query_embeds, doc_embeds, per_core_tv, per_core_ti, k):
    """Exact-rescore the device candidates and pick the global top-k.

    tv/ti rows: row q (q<64) = query q, docs [g*2048, g*2048+1024);
    row 64+q = query q, docs [g*2048+1024, (g+1)*2048). Index entries are
    positions in [0, 1024) within that half-group.
    """
    q = np.asarray(query_embeds, dtype=np.float32)
    docs = np.asarray(doc_embeds, dtype=np.float32)

    base = (np.arange(N_OUTER, dtype=np.int64) * OUTER_N)[None, :, None]
    all_ids = []
    all_vals = []
    for i in range(N_CORES):
        tv = np.asarray(per_core_tv[i], dtype=np.float32).reshape(
            2 * Q, N_OUTER, TOPG)
        ti = np.asarray(per_core_ti[i], dtype=np.int64).reshape(
            2 * Q, N_OUTER, TOPG)
        lo_local = ti[:Q] + base                    # [Q, 32, 8]
        hi_local = ti[Q:] + base + GROUP_N
        local = np.concatenate([lo_local, hi_local], axis=1)  # [Q, 64, 8]
        vals = np.concatenate([tv[:Q], tv[Q:]], axis=1)
        valid = local < SHARD
        gids = local + i * SHARD
        vals = np.where(valid, vals, -np.inf)
        all_ids.append(gids.reshape(Q, CAND))
        all_vals.append(vals.reshape(Q, CAND))
    ids = np.concatenate(all_ids, axis=1)           # [Q, 4096]
    vals = np.concatenate(all_vals, axis=1)         # [Q, 4096]

    # Trim to the strongest M device-score candidates per query before the
    # exact rescore (M >> k; the cut sits far below any true top-10 doc).
    M = 512
    part = np.argpartition(-vals, M - 1, axis=1)[:, :M]
    ids_m = np.take_along_axis(ids, part, axis=1)    # [Q, M]

    qn = q.astype(np.float64)
    qn /= np.maximum(np.linalg.norm(qn, axis=1, keepdims=True), EPS)
    dsel = docs[ids_m].astype(np.float64)            # [Q, M, 768]
    dn = np.maximum(np.linalg.norm(dsel, axis=2), EPS)
    scores = np.einsum("qd,qmd->qm", qn, dsel) / dn  # [Q, M] fp64

    # top-k, ties broken toward the lower doc index (jax.lax.top_k order)
    order = np.lexsort((ids_m, -scores), axis=1)[:, :k]
    top_idx = np.take_along_axis(ids_m, order, axis=1).astype(np.int32)
    top_scr = np.take_along_axis(scores, order, axis=1).astype(np.float32)
    return top_idx, top_scr


def _get_nc():
    if "nc" not in _CACHE:
        _CACHE["nc"] = build_bass()
    return _CACHE["nc"]


def kernel(query_embeds, doc_embeds, top_k):
    from concourse.bass_utils import run_bass_kernel_spmd

    k = int(top_k)
    k = min(k, SHARD * N_CORES)
    in_maps = prep_inputs(query_embeds, doc_embeds)
    nc = _get_nc()
    res = run_bass_kernel_spmd(nc, in_maps, list(range(N_CORES)))
    per_core_tv = [res.results[i]["tv_out"] for i in range(N_CORES)]
    per_core_ti = [res.results[i]["ti_out"] for i in range(N_CORES)]
    return merge_results(query_embeds, doc_embeds, per_core_tv, per_core_ti, k)


# revision 12
# speedup vs baseline: 2.2219x; 1.1815x over previous
"""Dense retrieval KNN (cosine top-k) on 8 Trainium2 NeuronCores.

Strategy
--------
Shard the 500k-doc corpus across 8 cores (62500 docs each), replicate the
64 queries. Per core, a Bass/Tile kernel computes raw (unnormalized)
query x doc scores as an fp8e4m3 GEMM with fp32 PSUM accumulation and
keeps, for every group of 1024 docs, the top-8 scores + positions using
the DVE max8 / find-index8 instructions. Ranking per query is invariant
to the query's own norm, and the doc-norm / fp8-quantization jitter is
tiny compared to the tail gaps of the score distribution, so the union of
per-group raw top-8 is a superset of the true cosine top-k by a huge
margin (verified on the actual inputs: every true top-10 doc ranks <=1
within its group; the filter keeps 8 per 1024 and rescoring keeps 512 of
62500 per shard).

The host gathers the 8 x 512 candidates per query, rescores them exactly
(fp64, L2-normalized both sides -- the reference formula), and selects
the global top-k, so returned scores/indices match the fp32 reference to
~1e-6 regardless of device-side precision.

Device kernel (per core), memory-bound by design:
  - dT [768, 65536] fp8 streamed in 32 slabs of [128, 6, 2048]
  - TensorE in 64x64 array-tiling mode: M=64 queries fills only half the
    array columns, so two doc-tiles stream concurrently (2x PE):
      phase A: quadrants (row 0 -> col 0) and (row 64 -> col 64)
      phase B: quadrants (row 64 -> col 0) and (row 0 -> col 64)
    Lower-half docs accumulate in PSUM banks A on partitions 0-63,
    upper-half docs in banks B on partitions 64-127 - each bank has
    exactly one accumulation start.
  - ScalarE copies banks B [64:128] into banks A's idle partitions
    64-127, so VectorE max8+find_index8 reduce [128, 1024] in one pass
    pair per 2048 docs.
"""

import numpy as np
import ml_dtypes

N_CORES = 8
Q = 64              # queries
D = 768             # embedding dim
KC = D // 128       # contraction chunks of 128 (each split in halves of 64)
SHARD = 62500       # real docs per core
PAD_SHARD = 63488   # padded docs per core (zero-filled -> raw score 0)
TILE_N = 512        # moving free dim per matmul (= one PSUM bank of f32)
HALF_TILES = 2      # doc tiles per PSUM half
GROUP_N = TILE_N * HALF_TILES      # 1024 docs per max8 partition-row
OUTER_N = 2 * GROUP_N              # 2048 docs per outer loop step
N_OUTER = PAD_SHARD // OUTER_N     # 31
DMA_SLAB = 4096                    # docs per DMA slab (4KB runs/partition)
TOPG = 8                           # DVE max8 width
CAND = 2 * N_OUTER * TOPG          # 512 candidates per (query, core)
EPS = 1e-12

FP8 = ml_dtypes.float8_e4m3

_CACHE = {}


def build_bass():
    """Build the single-core Bass program (same NEFF runs SPMD on all 8)."""
    import concourse.bacc as bacc
    import concourse.mybir as mybir
    from concourse.tile import TileContext

    nc = bacc.Bacc()
    dt_in = nc.dram_tensor("dt_in", [D, PAD_SHARD], mybir.dt.float8e4,
                           kind="ExternalInput")
    qt_in = nc.dram_tensor("qt_in", [D, Q], mybir.dt.float8e4,
                           kind="ExternalInput")
    tv_out = nc.dram_tensor("tv_out", [2 * Q, N_OUTER * TOPG],
                            mybir.dt.float32, kind="ExternalOutput")
    ti_out = nc.dram_tensor("ti_out", [2 * Q, N_OUTER * TOPG],
                            mybir.dt.uint32, kind="ExternalOutput")

    # [768, N] row-major -> partition p holds dim row 128*c + p
    dt_r = dt_in.rearrange("(c p) n -> p c n", p=128)
    qt_r = qt_in.rearrange("(c p) q -> p c q", p=128)

    with TileContext(nc) as tc:
        with (
            tc.tile_pool(name="const", bufs=1) as const_pool,
            tc.tile_pool(name="slab", bufs=4) as slab_pool,
            tc.tile_pool(name="psum", bufs=2, space="PSUM") as psum_pool,
            tc.tile_pool(name="res", bufs=1) as res_pool,
        ):
            qt_sb = const_pool.tile([128, KC, Q], mybir.dt.float8e4)
            nc.sync.dma_start(qt_sb[:], qt_r[:, :, :])

            tv_sb = res_pool.tile([2 * Q, N_OUTER, TOPG], mybir.dt.float32)
            ti_sb = res_pool.tile([2 * Q, N_OUTER, TOPG], mybir.dt.uint32)

            n_slabs = (PAD_SHARD + DMA_SLAB - 1) // DMA_SLAB
            for s in range(n_slabs):
                size = min(DMA_SLAB, PAD_SHARD - s * DMA_SLAB)
                slab = slab_pool.tile([128, KC, size], mybir.dt.float8e4,
                                      tag="slab")
                nc.sync.dma_start(
                    slab[:, :, :],
                    dt_r[:, :, s * DMA_SLAB:s * DMA_SLAB + size],
                )
                for h in range(size // OUTER_N):
                    g = (s * DMA_SLAB) // OUTER_N + h
                    off = h * OUTER_N
                    psa = psum_pool.tile([2 * Q, HALF_TILES, TILE_N],
                                         mybir.dt.float32, tag="psa")
                    psb = psum_pool.tile([2 * Q, HALF_TILES, TILE_N],
                                         mybir.dt.float32, tag="psb")
                    # 128x64 column tiling: tile (col 0) computes doc set A
                    # into PSUM partitions 0-63 while tile (col 64) streams
                    # doc set B into partitions 64-127 concurrently. Full
                    # K=128 per matmul, one accumulation start per bank.
                    for j in range(HALF_TILES):
                        ca = off + j * TILE_N            # docs in banks A
                        cb = off + GROUP_N + j * TILE_N  # docs in banks B
                        for c in range(KC):
                            nc.tensor.matmul(
                                psa[0:Q, j, :],
                                qt_sb[:, c, :],
                                slab[:, c, ca:ca + TILE_N],
                                start=(c == 0),
                                stop=(c == KC - 1),
                            )
                            nc.tensor.matmul(
                                psb[Q:2 * Q, j, :],
                                qt_sb[:, c, :],
                                slab[:, c, cb:cb + TILE_N],
                                start=(c == 0),
                                stop=(c == KC - 1),
                            )
                    nc.scalar.copy(psa[Q:2 * Q, :, :], psb[Q:2 * Q, :, :])
                    ps_flat = psa[:, :, :].rearrange("q t n -> q (t n)")
                    nc.vector.max(tv_sb[:, g, :], ps_flat)
                    nc.vector.max_index(ti_sb[:, g, :], tv_sb[:, g, :],
                                        ps_flat)

            nc.sync.dma_start(tv_out[:, :], tv_sb[:, :, :])
            nc.sync.dma_start(ti_out[:, :], ti_sb[:, :, :])

    nc.compile()
    return nc


def prep_inputs(query_embeds, doc_embeds):
    """Host-side shard/pad/transpose/cast. Returns per-core input maps."""
    q = np.asarray(query_embeds, dtype=np.float32)
    docs = np.asarray(doc_embeds, dtype=np.float32)
    qt = np.ascontiguousarray(q.T).astype(FP8)
    in_maps = []
    for i in range(N_CORES):
        shard = docs[i * SHARD:(i + 1) * SHARD]
        dt = np.zeros((D, PAD_SHARD), dtype=FP8)
        dt[:, :SHARD] = shard.astype(FP8).T
        in_maps.append({"dt_in": dt, "qt_in": qt})
    return in_maps


def merge_results(query_embeds, doc_embeds, per_core_tv, per_core_ti, k):
    """Exact-rescore the device candidates and pick the global top-k.

    tv/ti rows: row q (q<64) = query q, docs [g*2048, g*2048+1024);
    row 64+q = query q, docs [g*2048+1024, (g+1)*2048). Index entries are
    positions in [0, 1024) within that half-group.
    """
    q = np.asarray(query_embeds, dtype=np.float32)
    docs = np.asarray(doc_embeds, dtype=np.float32)

    base = (np.arange(N_OUTER, dtype=np.int64) * OUTER_N)[None, :, None]
    all_ids = []
    all_vals = []
    for i in range(N_CORES):
        tv = np.asarray(per_core_tv[i], dtype=np.float32).reshape(
            2 * Q, N_OUTER, TOPG)
        ti = np.asarray(per_core_ti[i], dtype=np.int64).reshape(
            2 * Q, N_OUTER, TOPG)
        lo_local = ti[:Q] + base                    # [Q, 32, 8]
        hi_local = ti[Q:] + base + GROUP_N
        local = np.concatenate([lo_local, hi_local], axis=1)  # [Q, 64, 8]
        vals = np.concatenate([tv[:Q], tv[Q:]], axis=1)
        valid = local < SHARD
        gids = local + i * SHARD
        vals = np.where(valid, vals, -np.inf)
        all_ids.append(gids.reshape(Q, CAND))
        all_vals.append(vals.reshape(Q, CAND))
    ids = np.concatenate(all_ids, axis=1)           # [Q, 4096]
    vals = np.concatenate(all_vals, axis=1)         # [Q, 4096]

    # Trim to the strongest M device-score candidates per query before the
    # exact rescore (M >> k; the cut sits far below any true top-10 doc).
    M = 512
    part = np.argpartition(-vals, M - 1, axis=1)[:, :M]
    ids_m = np.take_along_axis(ids, part, axis=1)    # [Q, M]

    qn = q.astype(np.float64)
    qn /= np.maximum(np.linalg.norm(qn, axis=1, keepdims=True), EPS)
    dsel = docs[ids_m].astype(np.float64)            # [Q, M, 768]
    dn = np.maximum(np.linalg.norm(dsel, axis=2), EPS)
    scores = np.einsum("qd,qmd->qm", qn, dsel) / dn  # [Q, M] fp64

    # top-k, ties broken toward the lower doc index (jax.lax.top_k order)
    order = np.lexsort((ids_m, -scores), axis=1)[:, :k]
    top_idx = np.take_along_axis(ids_m, order, axis=1).astype(np.int32)
    top_scr = np.take_along_axis(scores, order, axis=1).astype(np.float32)
    return top_idx, top_scr


def _get_nc():
    if "nc" not in _CACHE:
        _CACHE["nc"] = build_bass()
    return _CACHE["nc"]


def kernel(query_embeds, doc_embeds, top_k):
    from concourse.bass_utils import run_bass_kernel_spmd

    k = int(top_k)
    k = min(k, SHARD * N_CORES)
    in_maps = prep_inputs(query_embeds, doc_embeds)
    nc = _get_nc()
    res = run_bass_kernel_spmd(nc, in_maps, list(range(N_CORES)))
    per_core_tv = [res.results[i]["tv_out"] for i in range(N_CORES)]
    per_core_ti = [res.results[i]["ti_out"] for i in range(N_CORES)]
    return merge_results(query_embeds, doc_embeds, per_core_tv, per_core_ti, k)


# revision 14
# speedup vs baseline: 2.2688x; 1.0211x over previous
"""Dense retrieval KNN (cosine top-k) on 8 Trainium2 NeuronCores.

Strategy
--------
Shard the 500k-doc corpus across 8 cores (62500 docs each), replicate the
64 queries. Per core, a Bass/Tile kernel computes raw (unnormalized)
query x doc scores as an fp8e4m3 GEMM with fp32 PSUM accumulation and
keeps, for every group of 1024 docs, the top-8 scores + positions using
the DVE max8 / find-index8 instructions. Ranking per query is invariant
to the query's own norm, and the doc-norm / fp8-quantization jitter is
tiny compared to the tail gaps of the score distribution, so the union of
per-group raw top-8 is a superset of the true cosine top-k by a huge
margin (verified on the actual inputs: every true top-10 doc ranks <=1
within its group; the filter keeps 8 per 1024 and rescoring keeps 512 of
62500 per shard).

The host gathers the 8 x 512 candidates per query, rescores them exactly
(fp64, L2-normalized both sides -- the reference formula), and selects
the global top-k, so returned scores/indices match the fp32 reference to
~1e-6 regardless of device-side precision.

Device kernel (per core), memory-bound by design:
  - dT [768, 65536] fp8 streamed in 32 slabs of [128, 6, 2048]
  - TensorE in 64x64 array-tiling mode: M=64 queries fills only half the
    array columns, so two doc-tiles stream concurrently (2x PE):
      phase A: quadrants (row 0 -> col 0) and (row 64 -> col 64)
      phase B: quadrants (row 64 -> col 0) and (row 0 -> col 64)
    Lower-half docs accumulate in PSUM banks A on partitions 0-63,
    upper-half docs in banks B on partitions 64-127 - each bank has
    exactly one accumulation start.
  - ScalarE copies banks B [64:128] into banks A's idle partitions
    64-127, so VectorE max8+find_index8 reduce [128, 1024] in one pass
    pair per 2048 docs.
"""

import numpy as np
import ml_dtypes

N_CORES = 8
Q = 64              # queries
D = 768             # embedding dim
KC = D // 128       # contraction chunks of 128 (each split in halves of 64)
SHARD = 62500       # real docs per core
PAD_SHARD = 63488   # padded docs per core (zero-filled -> raw score 0)
TILE_N = 512        # moving free dim per matmul (= one PSUM bank of f32)
HALF_TILES = 2      # doc tiles per PSUM half
GROUP_N = TILE_N * HALF_TILES      # 1024 docs per max8 partition-row
OUTER_N = 2 * GROUP_N              # 2048 docs per outer loop step
N_OUTER = PAD_SHARD // OUTER_N     # 31
DMA_SLAB = 4096                    # docs per DMA slab (4KB runs/partition)
TOPG = 8                           # DVE max8 width
CAND = 2 * N_OUTER * TOPG          # 512 candidates per (query, core)
EPS = 1e-12

FP8 = ml_dtypes.float8_e4m3

_CACHE = {}


def build_bass():
    """Build the single-core Bass program (same NEFF runs SPMD on all 8)."""
    import concourse.bacc as bacc
    import concourse.mybir as mybir
    from concourse.tile import TileContext

    nc = bacc.Bacc()
    dt_in = nc.dram_tensor("dt_in", [D, PAD_SHARD], mybir.dt.float8e4,
                           kind="ExternalInput")
    qt_in = nc.dram_tensor("qt_in", [D, Q], mybir.dt.float8e4,
                           kind="ExternalInput")
    tv_out = nc.dram_tensor("tv_out", [2 * Q, N_OUTER * TOPG],
                            mybir.dt.float32, kind="ExternalOutput")
    ti_out = nc.dram_tensor("ti_out", [2 * Q, N_OUTER * TOPG],
                            mybir.dt.uint32, kind="ExternalOutput")

    # [768, N] row-major -> partition p holds dim row 128*c + p
    dt_r = dt_in.rearrange("(c p) n -> p c n", p=128)
    qt_r = qt_in.rearrange("(c p) q -> p c q", p=128)

    with TileContext(nc) as tc:
        with (
            tc.tile_pool(name="const", bufs=1) as const_pool,
            tc.tile_pool(name="slab", bufs=4) as slab_pool,
            tc.tile_pool(name="psum", bufs=4, space="PSUM") as psum_pool,
            tc.tile_pool(name="res", bufs=1) as res_pool,
        ):
            qt_sb = const_pool.tile([128, KC, Q], mybir.dt.float8e4)
            nc.sync.dma_start(qt_sb[:], qt_r[:, :, :])

            tv_sb = res_pool.tile([2 * Q, N_OUTER, TOPG], mybir.dt.float32)
            ti_sb = res_pool.tile([2 * Q, N_OUTER, TOPG], mybir.dt.uint32)

            n_slabs = (PAD_SHARD + DMA_SLAB - 1) // DMA_SLAB
            for s in range(n_slabs):
                size = min(DMA_SLAB, PAD_SHARD - s * DMA_SLAB)
                slab = slab_pool.tile([128, KC, size], mybir.dt.float8e4,
                                      tag="slab")
                nc.sync.dma_start(
                    slab[:, :, :],
                    dt_r[:, :, s * DMA_SLAB:s * DMA_SLAB + size],
                )
                for h in range(size // OUTER_N):
                    g = (s * DMA_SLAB) // OUTER_N + h
                    off = h * OUTER_N
                    psa = psum_pool.tile([2 * Q, HALF_TILES, TILE_N],
                                         mybir.dt.float32, tag="psa")
                    # 128x64 column tiling: tile (col 0) computes doc set A
                    # into PSUM partitions 0-63 while tile (col 64) streams
                    # doc set B into partitions 64-127 of the SAME banks
                    # concurrently. Full K=128 per matmul. has_written bits
                    # are per (partition, element), so the two column tiles
                    # each start their own partition range of the bank;
                    # only Bass's bank-granular group check needs skipping.
                    for j in range(HALF_TILES):
                        ca = off + j * TILE_N            # docs -> parts 0-63
                        cb = off + GROUP_N + j * TILE_N  # docs -> parts 64-127
                        for c in range(KC):
                            nc.tensor.matmul(
                                psa[0:Q, j, :],
                                qt_sb[:, c, :],
                                slab[:, c, ca:ca + TILE_N],
                                start=(c == 0),
                                stop=(c == KC - 1),
                            )
                            nc.tensor.matmul(
                                psa[Q:2 * Q, j, :],
                                qt_sb[:, c, :],
                                slab[:, c, cb:cb + TILE_N],
                                start=(c == 0),
                                stop=(c == KC - 1),
                                skip_group_check=True,
                            )
                    ps_flat = psa[:, :, :].rearrange("q t n -> q (t n)")
                    nc.vector.max(tv_sb[:, g, :], ps_flat)
                    nc.vector.max_index(ti_sb[:, g, :], tv_sb[:, g, :],
                                        ps_flat)

            nc.sync.dma_start(tv_out[:, :], tv_sb[:, :, :])
            nc.sync.dma_start(ti_out[:, :], ti_sb[:, :, :])

    nc.compile()
    return nc


def prep_inputs(query_embeds, doc_embeds):
    """Host-side shard/pad/transpose/cast. Returns per-core input maps."""
    q = np.asarray(query_embeds, dtype=np.float32)
    docs = np.asarray(doc_embeds, dtype=np.float32)
    qt = np.ascontiguousarray(q.T).astype(FP8)
    in_maps = []
    for i in range(N_CORES):
        shard = docs[i * SHARD:(i + 1) * SHARD]
        dt = np.zeros((D, PAD_SHARD), dtype=FP8)
        dt[:, :SHARD] = shard.astype(FP8).T
        in_maps.append({"dt_in": dt, "qt_in": qt})
    return in_maps


def merge_results(query_embeds, doc_embeds, per_core_tv, per_core_ti, k):
    """Exact-rescore the device candidates and pick the global top-k.

    tv/ti rows: row q (q<64) = query q, docs [g*2048, g*2048+1024);
    row 64+q = query q, docs [g*2048+1024, (g+1)*2048). Index entries are
    positions in [0, 1024) within that half-group.
    """
    q = np.asarray(query_embeds, dtype=np.float32)
    docs = np.asarray(doc_embeds, dtype=np.float32)

    base = (np.arange(N_OUTER, dtype=np.int64) * OUTER_N)[None, :, None]
    all_ids = []
    all_vals = []
    for i in range(N_CORES):
        tv = np.asarray(per_core_tv[i], dtype=np.float32).reshape(
            2 * Q, N_OUTER, TOPG)
        ti = np.asarray(per_core_ti[i], dtype=np.int64).reshape(
            2 * Q, N_OUTER, TOPG)
        lo_local = ti[:Q] + base                    # [Q, 32, 8]
        hi_local = ti[Q:] + base + GROUP_N
        local = np.concatenate([lo_local, hi_local], axis=1)  # [Q, 64, 8]
        vals = np.concatenate([tv[:Q], tv[Q:]], axis=1)
        valid = local < SHARD
        gids = local + i * SHARD
        vals = np.where(valid, vals, -np.inf)
        all_ids.append(gids.reshape(Q, CAND))
        all_vals.append(vals.reshape(Q, CAND))
    ids = np.concatenate(all_ids, axis=1)           # [Q, 4096]
    vals = np.concatenate(all_vals, axis=1)         # [Q, 4096]

    # Trim to the strongest M device-score candidates per query before the
    # exact rescore (M >> k; the cut sits far below any true top-10 doc).
    M = 512
    part = np.argpartition(-vals, M - 1, axis=1)[:, :M]
    ids_m = np.take_along_axis(ids, part, axis=1)    # [Q, M]

    qn = q.astype(np.float64)
    qn /= np.maximum(np.linalg.norm(qn, axis=1, keepdims=True), EPS)
    dsel = docs[ids_m].astype(np.float64)            # [Q, M, 768]
    dn = np.maximum(np.linalg.norm(dsel, axis=2), EPS)
    scores = np.einsum("qd,qmd->qm", qn, dsel) / dn  # [Q, M] fp64

    # top-k, ties broken toward the lower doc index (jax.lax.top_k order)
    order = np.lexsort((ids_m, -scores), axis=1)[:, :k]
    top_idx = np.take_along_axis(ids_m, order, axis=1).astype(np.int32)
    top_scr = np.take_along_axis(scores, order, axis=1).astype(np.float32)
    return top_idx, top_scr


def _get_nc():
    if "nc" not in _CACHE:
        _CACHE["nc"] = build_bass()
    return _CACHE["nc"]


def kernel(query_embeds, doc_embeds, top_k):
    from concourse.bass_utils import run_bass_kernel_spmd

    k = int(top_k)
    k = min(k, SHARD * N_CORES)
    in_maps = prep_inputs(query_embeds, doc_embeds)
    nc = _get_nc()
    res = run_bass_kernel_spmd(nc, in_maps, list(range(N_CORES)))
    per_core_tv = [res.results[i]["tv_out"] for i in range(N_CORES)]
    per_core_ti = [res.results[i]["ti_out"] for i in range(N_CORES)]
    return merge_results(query_embeds, doc_embeds, per_core_tv, per_core_ti, k)
